# revision 33
# baseline (speedup 1.0000x reference)
"""Causal self-attention Trainium2 kernel (8 NeuronCores).

Sharding: core = b*2 + g where b = batch (4), g = head-group (2 groups x 8 heads).
Each core computes, for its (batch, head-group):
    qkv = x[b] @ w_attn[:, group cols] + b_attn[group]
    y_g = softmax_causal(q k^T / sqrt(hs)) v          (8 heads)
    part = y_g @ w_proj[group rows, :] (+ b_proj on g==0)
Host sums the two per-batch partials (the c_proj row-split reduction).

Key device-side structure (v2, rewritten from the DRAM-roundtrip baseline):
  - x arrives from the host already transposed and cast to bf16 (xT [C, T]),
    so the qkv matmuls start ~1.5us in; no on-device cast/store/XBAR-transpose
    pipeline, no cold-clock stalls from a starved PE.
  - Score matmuls have contraction = head_size = 64, i.e. half the PE array.
    The head PAIR (2hp, 2hp+1) lives at partitions 0-63 / 64-127 of the same
    qkT tile, so the two matmuls are issued back-to-back: they land on
    disjoint PE row-groups (tile_position (0,0)/(64,0) auto-derived) and run
    concurrently -- ~2x on the score phase.
  - Causal trim: for diagonal blocks (d = ki - 4*qj >= 0) the scores / exp /
    mask / AV ops only cover q-columns [d*128, 512), and the mask multiply is
    a single [128,2,128] triangle.
  - Softmax denominator: v is extended with a ones column (col 64), so the av
    matmul's psum row 64 accumulates sum_k exp. Normalization is deferred:
    reciprocal per head is computed on the cs tiles, broadcast across
    partitions with a small SBUF->SBUF DMA, applied with one multiply per
    (head-pair, q-block).
  - Emission interleaves qkv(n+1) / proj(qj-1) psum-groups between attention
    hp-chunks so the in-order PE queue always has fill work while exp (ACT)
    chases the score matmuls.
  - PSUM: scores 2x[128,2,512] (4 banks) + AV py0/py1 (2 banks) + one shared
    qkv/proj pool (2 banks) = 8 banks.
"""

import sys

sys.path.insert(0, "/opt/trn_rl_repo")

import math
import numpy as np
import ml_dtypes

import concourse.bass as bass
import concourse.bacc as bacc
import concourse.tile as tile
from concourse import mybir
from concourse import bass_utils

def _ensure_ntff_hook():
    """Provide antenv.axon_hooks (NTFF profiling registry) if the image's
    antenv lacks it, wiring the ctypes-based hook from trn_agent_boot."""
    import types
    try:
        import antenv.axon_hooks  # noqa: F401
        return
    except ImportError:
        pass
    try:
        import antenv
        from trn_agent_boot.trn_boot import _ntff_profile_via_ctypes
        hook = _ntff_profile_via_ctypes("/opt/axon/libaxon_pjrt.so")
    except Exception:
        return
    mod = types.ModuleType("antenv.axon_hooks")
    mod.get_axon_ntff_profile_hook = lambda: hook
    mod.set_axon_ntff_profile_hook = lambda h: None
    sys.modules["antenv.axon_hooks"] = mod
    antenv.axon_hooks = mod


_ensure_ntff_hook()

F32 = mybir.dt.float32
BF16 = mybir.dt.bfloat16
AF = mybir.ActivationFunctionType
ALU = mybir.AluOpType

T = 2048
C = 1024
HS = 64           # head size
NHL = 8           # heads per core
GC = NHL * HS     # 512: group width
CK = C // 128     # 8 contraction tiles for qkv
MT = T // 128     # 16 row tiles
QB = 512          # q block (one fp32 PSUM bank)
NQ = T // QB      # 4
SCALE = 1.0 / math.sqrt(HS)
N_CORES = 8


def build_program():
    nc = bacc.Bacc("TRN2", target_bir_lowering=False, debug=False, num_devices=N_CORES)
    xT_d = nc.dram_tensor("xT", [C, T], BF16, kind="ExternalInput").ap()
    wqkv_d = nc.dram_tensor("w_qkv", [C, 3 * GC], BF16, kind="ExternalInput").ap()
    bqk_d = nc.dram_tensor("b_qk", [128, 8], F32, kind="ExternalInput").ap()
    bv_d = nc.dram_tensor("b_v", [GC], F32, kind="ExternalInput").ap()
    wproj_d = nc.dram_tensor("w_proj", [GC, C], BF16, kind="ExternalInput").ap()
    bproj_d = nc.dram_tensor("b_proj", [C], F32, kind="ExternalInput").ap()
    mask2_d = nc.dram_tensor("mask2", [128, 2, 128], BF16, kind="ExternalInput").ap()
    y_d = nc.dram_tensor("y", [T, C], F32, kind="ExternalOutput").ap()

    def bcast(ap, parts):
        # replicate a [1, N] slice across `parts` partitions (DMA source AP)
        return bass.AP(tensor=ap.tensor, offset=ap.offset, ap=[[0, parts]] + list(ap.ap)[-1:])

    with tile.TileContext(nc) as tc:
        from contextlib import ExitStack

        with ExitStack() as ctx:
            const = ctx.enter_context(tc.tile_pool(name="const", bufs=1))
            dram = ctx.enter_context(tc.tile_pool(name="dram", bufs=1, space="DRAM"))

            # ---------------- constants + input loads ----------------
            b_qk = const.tile([128, 8], F32)
            nc.scalar.dma_start(out=b_qk, in_=bqk_d)
            mask2 = const.tile([128, 2, 128], BF16)
            nc.sync.dma_start(out=mask2, in_=mask2_d)

            xT = const.tile([128, CK, T], BF16)
            xr = xT_d.rearrange("(c p) t -> p c t", p=128)
            w_qkv = const.tile([128, CK, 3 * GC], BF16)
            wr = wqkv_d.rearrange("(c p) n -> p c n", p=128)
            # first q-block's xT chunks and the w chunks split across both
            # queues so qkv(0)'s m-groups (which need every c-chunk of w) can
            # finish ~5us after DMA starts; the rest of xT follows as wide
            # (3KB-line) chunks
            for c in range(CK):
                qa, qb = (nc.scalar, nc.sync) if c % 2 == 0 else (nc.sync, nc.scalar)
                qb.dma_start(out=xT[:, c, 0:QB], in_=xr[:, c, 0:QB])
                qa.dma_start(out=w_qkv[:, c, :], in_=wr[:, c, :])
            for c in range(CK):
                q = nc.sync if c % 2 == 0 else nc.scalar
                q.dma_start(out=xT[:, c, QB:T], in_=xr[:, c, QB:T])

            b_v = const.tile([128, GC], F32)
            nc.scalar.dma_start(out=b_v, in_=bcast(bv_d, 128))
            w_proj = const.tile([128, 4, C], BF16)
            nc.scalar.dma_start(out=w_proj, in_=wproj_d.rearrange("(c p) n -> p c n", p=128))
            b_proj = const.tile([128, C], F32)
            nc.scalar.dma_start(out=b_proj, in_=bcast(bproj_d, 128))

            # ---------------- persistent tiles ----------------
            # qkT rows: m 0..3 = q cols (head-pair m at partitions 0-63 /
            # 64-127), m 4..7 = k cols.
            qkT = const.tile([128, 8, T], BF16)
            # v padded to 128 stationary columns: [v (64) | ones | zeros] so
            # the AV ldweights takes the fast-weight-load path (128 cols);
            # psum row 64 accumulates sum_k exp, rows 65-127 accumulate zeros
            v2 = const.tile([128, MT, NHL, 128], BF16)
            nc.vector.memset(v2[:, :, :, HS:128], 0.0)
            nc.vector.memset(v2[:, :, :, HS:HS + 1], 1.0)
            yTu = const.tile([128, 4, T], BF16)   # unnormalized y^T
            den_dram = dram.tile([8, T], F32)

            with tc.tile_pool(name="mm", bufs=2, space="PSUM") as pmm, \
                 tc.tile_pool(name="pys", bufs=1, space="PSUM") as pys, \
                 tc.tile_pool(name="pss", bufs=2, space="PSUM") as pss, \
                 tc.tile_pool(name="sexp", bufs=6) as sexp, \
                 tc.tile_pool(name="nbc", bufs=2) as nbc, \
                 tc.tile_pool(name="ost", bufs=3) as ostage:

                def qkv_m_group(n, m):
                    ps = pmm.tile([128, QB], F32, tag="mm")
                    for c in range(CK):
                        nc.tensor.matmul(ps,
                                         lhsT=w_qkv[:, c, m * 128:(m + 1) * 128],
                                         rhs=xT[:, c, n * QB:(n + 1) * QB],
                                         start=(c == 0), stop=(c == CK - 1))
                    nc.vector.tensor_scalar_add(out=qkT[:, m, n * QB:(n + 1) * QB],
                                                in0=ps, scalar1=b_qk[:, m:m + 1])

                def qkv_t_group(t):
                    ps = pmm.tile([128, QB], F32, tag="mm")
                    for c in range(CK):
                        nc.tensor.matmul(ps,
                                         lhsT=xT[:, c, t * 128:(t + 1) * 128],
                                         rhs=w_qkv[:, c, 2 * GC:3 * GC],
                                         start=(c == 0), stop=(c == CK - 1))
                    nc.vector.tensor_tensor(out=v2[:, t, :, 0:HS],
                                            in0=ps.rearrange("p (h d) -> p h d", d=HS),
                                            in1=b_v.rearrange("p (h d) -> p h d", d=HS),
                                            op=ALU.add)

                def qkv_groups(n):
                    gs = [(lambda n=n, m=m: qkv_m_group(n, m)) for m in range(8)]
                    gs += [(lambda t=t: qkv_t_group(t)) for t in range(4 * n, 4 * n + 4)]
                    return gs

                def proj_group(t, n2):
                    ps = pmm.tile([128, QB], F32, tag="mm")
                    for c4 in range(4):
                        nc.tensor.matmul(ps,
                                         lhsT=yTu[:, c4, t * 128:(t + 1) * 128],
                                         rhs=w_proj[:, c4, n2 * QB:(n2 + 1) * QB],
                                         start=(c4 == 0), stop=(c4 == 3))
                    ot = ostage.tile([128, QB], F32, tag="ot")
                    nc.vector.tensor_tensor(out=ot, in0=ps,
                                            in1=b_proj[:, n2 * QB:(n2 + 1) * QB],
                                            op=ALU.add)
                    eng = nc.sync if (2 * t + n2) % 2 == 0 else nc.scalar
                    eng.dma_start(out=y_d[t * 128:(t + 1) * 128, n2 * QB:(n2 + 1) * QB],
                                  in_=ot)

                def proj_groups(qj):
                    return [(lambda t=t, n2=n2: proj_group(t, n2))
                            for t in range(4 * qj, 4 * qj + 4) for n2 in range(2)]

                def attn_chunk(qj, hp):
                    nki = 4 * (qj + 1)
                    qcols = slice(qj * QB, (qj + 1) * QB)
                    py0 = pys.tile([128, QB], F32, tag="py0")
                    py1 = pys.tile([128, QB], F32, tag="py1")
                    for ki in range(nki):
                        d = ki - 4 * qj
                        q0 = max(d, 0) * 128
                        cols = slice(qj * QB + q0, (qj + 1) * QB)
                        ps = pss.tile([128, 2, QB], F32, tag="s")
                        # head pair on disjoint PE row-groups -> concurrent
                        nc.tensor.matmul(ps[:, 0, q0:],
                                         lhsT=qkT[0:HS, 4 + hp, ki * 128:(ki + 1) * 128],
                                         rhs=qkT[0:HS, hp, cols],
                                         start=True, stop=True)
                        nc.tensor.matmul(ps[:, 1, q0:],
                                         lhsT=qkT[HS:128, 4 + hp, ki * 128:(ki + 1) * 128],
                                         rhs=qkT[HS:128, hp, cols],
                                         start=True, stop=True)
                        ex = sexp.tile([128, 2, QB], BF16, tag="e")
                        nc.scalar.activation(out=ex[:, :, q0:], in_=ps[:, :, q0:],
                                             func=AF.Exp, scale=SCALE)
                        if d >= 0:
                            nc.vector.tensor_tensor(out=ex[:, :, q0:q0 + 128],
                                                    in0=ex[:, :, q0:q0 + 128],
                                                    in1=mask2, op=ALU.mult)
                        nc.tensor.matmul(py0[:, q0:], lhsT=v2[:, ki, 2 * hp, :],
                                         rhs=ex[:, 0, q0:],
                                         start=(ki == 0), stop=(ki == nki - 1))
                        nc.tensor.matmul(py1[:, q0:], lhsT=v2[:, ki, 2 * hp + 1, :],
                                         rhs=ex[:, 1, q0:],
                                         start=(ki == 0), stop=(ki == nki - 1))
                    # drain: y rows (0..63) to yTu, colsum rows (64) to
                    # single-row staging tiles
                    cs0 = nbc.tile([1, QB], F32, tag="cs0")
                    cs1 = nbc.tile([1, QB], F32, tag="cs1")
                    nc.vector.tensor_copy(out=yTu[0:HS, hp, qcols], in_=py0[0:HS, :])
                    nc.scalar.copy(out=yTu[HS:128, hp, qcols], in_=py1[0:HS, :])
                    nc.scalar.copy(out=cs0, in_=py0[HS:HS + 1, :])
                    nc.scalar.copy(out=cs1, in_=py1[HS:HS + 1, :])
                    return cs0, cs1

                def normalize_hp(qj, hp, cs0, cs1):
                    # reciprocal per head-pair, bounced through DRAM on the
                    # otherwise-idle SWDGE queue to broadcast across
                    # partitions (same-queue FIFO orders store->load)
                    qcols = slice(qj * QB, (qj + 1) * QB)
                    h0, h1 = 2 * hp, 2 * hp + 1
                    rc0 = nbc.tile([1, QB], F32, tag="rc0")
                    rc1 = nbc.tile([1, QB], F32, tag="rc1")
                    nc.vector.reciprocal_approx_fast(out=rc0, in_=cs0)
                    nc.vector.reciprocal_approx_fast(out=rc1, in_=cs1)
                    nc.gpsimd.dma_start(out=den_dram[h0:h0 + 1, qcols], in_=rc0)
                    nc.gpsimd.dma_start(out=den_dram[h1:h1 + 1, qcols], in_=rc1)
                    rb = nbc.tile([128, QB], F32, tag="rb")
                    nc.gpsimd.dma_start(out=rb[0:HS, :],
                                        in_=bcast(den_dram[h0:h0 + 1, qcols], HS))
                    nc.gpsimd.dma_start(out=rb[HS:128, :],
                                        in_=bcast(den_dram[h1:h1 + 1, qcols], HS))
                    nc.vector.tensor_tensor(out=yTu[:, hp, qcols],
                                            in0=yTu[:, hp, qcols],
                                            in1=rb, op=ALU.mult)

                # ---------------- interleaved emission ----------------
                for g in qkv_groups(0):
                    g()
                fillers = {
                    0: qkv_groups(1),
                    1: qkv_groups(2) + proj_groups(0),
                    2: qkv_groups(3) + proj_groups(1),
                    3: proj_groups(2),
                }
                for qj in range(NQ):
                    fl = fillers[qj]
                    k = 0
                    for hp in range(4):
                        cs0, cs1 = attn_chunk(qj, hp)
                        normalize_hp(qj, hp, cs0, cs1)
                        take = (len(fl) * (hp + 1)) // 4 - (len(fl) * hp) // 4
                        for _ in range(take):
                            fl[k]()
                            k += 1
                for g in proj_groups(3):
                    g()

    nc.compile()
    return nc


def make_mask2():
    kk = np.arange(128)[:, None]
    qq = np.arange(128)[None, :]
    m = (qq >= kk).astype(ml_dtypes.bfloat16)
    return np.ascontiguousarray(np.broadcast_to(m[:, None, :], (128, 2, 128)))


def make_in_maps(x, w_attn, b_attn, w_proj, b_proj):
    mask2 = make_mask2()
    in_maps = []
    for core in range(N_CORES):
        b, g = core // 2, core % 2
        cq = slice(g * GC, (g + 1) * GC)
        ck = slice(C + g * GC, C + (g + 1) * GC)
        cv = slice(2 * C + g * GC, 2 * C + (g + 1) * GC)
        w_qkv_g = np.concatenate([w_attn[:, cq], w_attn[:, ck], w_attn[:, cv]], axis=1)
        in_maps.append({
            "xT": np.ascontiguousarray(
                np.asarray(x[b], dtype=np.float32).astype(ml_dtypes.bfloat16).T),
            "w_qkv": np.ascontiguousarray(w_qkv_g.astype(ml_dtypes.bfloat16)),
            # pre-tiled [128, 8]: b_qk[p, m] = flat[m*128 + p] (contiguous DMA)
            "b_qk": np.ascontiguousarray(
                np.concatenate([b_attn[cq], b_attn[ck]]).astype(np.float32)
                .reshape(8, 128).T),
            "b_v": np.ascontiguousarray(b_attn[cv]).astype(np.float32),
            "w_proj": np.ascontiguousarray(w_proj[g * GC:(g + 1) * GC, :].astype(ml_dtypes.bfloat16)),
            "b_proj": (b_proj if g == 0 else np.zeros_like(b_proj)).astype(np.float32),
            "mask2": mask2,
        })
    return in_maps


_PROGRAM = None


def kernel(x, w_attn, b_attn, w_proj, b_proj, _trace=False):
    global _PROGRAM
    x = np.asarray(x)
    B = x.shape[0]
    if _PROGRAM is None:
        _PROGRAM = build_program()
    nc = _PROGRAM
    in_maps = make_in_maps(x, np.asarray(w_attn), np.asarray(b_attn),
                           np.asarray(w_proj), np.asarray(b_proj))
    res = bass_utils.run_bass_kernel_spmd(nc, in_maps, core_ids=list(range(N_CORES)),
                                          trace=_trace)
    y = np.zeros((B, T, C), np.float32)
    for b in range(B):
        y[b] = res.results[2 * b]["y"] + res.results[2 * b + 1]["y"]
    if _trace:
        return y, res
    return y


# revision 36
# speedup vs baseline: 1.0394x; 1.0394x over previous
"""Causal self-attention Trainium2 kernel (8 NeuronCores).

Sharding: core = b*2 + g where b = batch (4), g = head-group (2 groups x 8 heads).
Each core computes, for its (batch, head-group):
    qkv = x[b] @ w_attn[:, group cols] + b_attn[group]
    y_g = softmax_causal(q k^T / sqrt(hs)) v          (8 heads)
    part = y_g @ w_proj[group rows, :] (+ b_proj on g==0)
Host sums the two per-batch partials (the c_proj row-split reduction).

Key device-side structure (v2, rewritten from the DRAM-roundtrip baseline):
  - x arrives from the host already transposed and cast to bf16 (xT [C, T]),
    so the qkv matmuls start ~1.5us in; no on-device cast/store/XBAR-transpose
    pipeline, no cold-clock stalls from a starved PE.
  - Score matmuls have contraction = head_size = 64, i.e. half the PE array.
    The head PAIR (2hp, 2hp+1) lives at partitions 0-63 / 64-127 of the same
    qkT tile, so the two matmuls are issued back-to-back: they land on
    disjoint PE row-groups (tile_position (0,0)/(64,0) auto-derived) and run
    concurrently -- ~2x on the score phase.
  - Causal trim: for diagonal blocks (d = ki - 4*qj >= 0) the scores / exp /
    mask / AV ops only cover q-columns [d*128, 512), and the mask multiply is
    a single [128,2,128] triangle.
  - Softmax denominator: v is extended with a ones column (col 64), so the av
    matmul's psum row 64 accumulates sum_k exp. Normalization is deferred:
    reciprocal per head is computed on the cs tiles, broadcast across
    partitions with a small SBUF->SBUF DMA, applied with one multiply per
    (head-pair, q-block).
  - Emission interleaves qkv(n+1) / proj(qj-1) psum-groups between attention
    hp-chunks so the in-order PE queue always has fill work while exp (ACT)
    chases the score matmuls.
  - PSUM: scores 2x[128,2,512] (4 banks) + AV py0/py1 (2 banks) + one shared
    qkv/proj pool (2 banks) = 8 banks.
"""

import sys

sys.path.insert(0, "/opt/trn_rl_repo")

import math
import numpy as np
import ml_dtypes

import concourse.bass as bass
import concourse.bacc as bacc
import concourse.tile as tile
from concourse import mybir
from concourse import bass_utils

def _ensure_ntff_hook():
    """Provide antenv.axon_hooks (NTFF profiling registry) if the image's
    antenv lacks it, wiring the ctypes-based hook from trn_agent_boot."""
    import types
    try:
        import antenv.axon_hooks  # noqa: F401
        return
    except ImportError:
        pass
    try:
        import antenv
        from trn_agent_boot.trn_boot import _ntff_profile_via_ctypes
        hook = _ntff_profile_via_ctypes("/opt/axon/libaxon_pjrt.so")
    except Exception:
        return
    mod = types.ModuleType("antenv.axon_hooks")
    mod.get_axon_ntff_profile_hook = lambda: hook
    mod.set_axon_ntff_profile_hook = lambda h: None
    sys.modules["antenv.axon_hooks"] = mod
    antenv.axon_hooks = mod


_ensure_ntff_hook()

F32 = mybir.dt.float32
BF16 = mybir.dt.bfloat16
AF = mybir.ActivationFunctionType
ALU = mybir.AluOpType

T = 2048
C = 1024
HS = 64           # head size
NHL = 8           # heads per core
GC = NHL * HS     # 512: group width
CK = C // 128     # 8 contraction tiles for qkv
MT = T // 128     # 16 row tiles
QB = 512          # q block (one fp32 PSUM bank)
NQ = T // QB      # 4
SCALE = 1.0 / math.sqrt(HS)
N_CORES = 8


def build_program():
    nc = bacc.Bacc("TRN2", target_bir_lowering=False, debug=False, num_devices=N_CORES)
    xT_d = nc.dram_tensor("xT", [C, T], BF16, kind="ExternalInput").ap()
    wqkv_d = nc.dram_tensor("w_qkv", [C, 3 * GC], BF16, kind="ExternalInput").ap()
    bqk_d = nc.dram_tensor("b_qk", [128, 8], F32, kind="ExternalInput").ap()
    bv_d = nc.dram_tensor("b_v", [GC], F32, kind="ExternalInput").ap()
    wproj_d = nc.dram_tensor("w_proj", [GC, C], BF16, kind="ExternalInput").ap()
    bproj_d = nc.dram_tensor("b_proj", [C], F32, kind="ExternalInput").ap()
    mask2_d = nc.dram_tensor("mask2", [128, 2, 128], BF16, kind="ExternalInput").ap()
    y_d = nc.dram_tensor("y", [T, C], F32, kind="ExternalOutput").ap()

    def bcast(ap, parts):
        # replicate a [1, N] slice across `parts` partitions (DMA source AP)
        return bass.AP(tensor=ap.tensor, offset=ap.offset, ap=[[0, parts]] + list(ap.ap)[-1:])

    with tile.TileContext(nc) as tc:
        from contextlib import ExitStack

        with ExitStack() as ctx:
            const = ctx.enter_context(tc.tile_pool(name="const", bufs=1))
            dram = ctx.enter_context(tc.tile_pool(name="dram", bufs=1, space="DRAM"))

            # ---------------- constants + input loads ----------------
            b_qk = const.tile([128, 8], F32)
            nc.scalar.dma_start(out=b_qk, in_=bqk_d)
            mask2 = const.tile([128, 2, 128], BF16)
            nc.sync.dma_start(out=mask2, in_=mask2_d)

            xT = const.tile([128, CK, T], BF16)
            xr = xT_d.rearrange("(c p) t -> p c t", p=128)
            w_qkv = const.tile([128, CK, 3 * GC], BF16)
            wr = wqkv_d.rearrange("(c p) n -> p c n", p=128)
            # first q-block's xT chunks and the w chunks split across both
            # queues so qkv(0)'s m-groups (which need every c-chunk of w) can
            # finish ~5us after DMA starts; the rest of xT follows as wide
            # (3KB-line) chunks
            for c in range(CK):
                qa, qb = (nc.scalar, nc.sync) if c % 2 == 0 else (nc.sync, nc.scalar)
                qb.dma_start(out=xT[:, c, 0:QB], in_=xr[:, c, 0:QB])
                qa.dma_start(out=w_qkv[:, c, :], in_=wr[:, c, :])
            for c in range(CK):
                q = nc.sync if c % 2 == 0 else nc.scalar
                q.dma_start(out=xT[:, c, QB:T], in_=xr[:, c, QB:T])

            b_v = const.tile([128, GC], F32)
            nc.scalar.dma_start(out=b_v, in_=bcast(bv_d, 128))
            w_proj = const.tile([128, 4, C], BF16)
            nc.scalar.dma_start(out=w_proj, in_=wproj_d.rearrange("(c p) n -> p c n", p=128))
            b_proj = const.tile([128, C], F32)
            nc.scalar.dma_start(out=b_proj, in_=bcast(bproj_d, 128))

            # ---------------- persistent tiles ----------------
            # qkT rows: m 0..3 = q cols (head-pair m at partitions 0-63 /
            # 64-127), m 4..7 = k cols.
            qkT = const.tile([128, 8, T], BF16)
            # v padded to 128 stationary columns: [v (64) | ones | zeros] so
            # the AV ldweights takes the fast-weight-load path (128 cols);
            # psum row 64 accumulates sum_k exp, rows 65-127 accumulate zeros
            v2 = const.tile([128, MT, NHL, 128], BF16)
            nc.vector.memset(v2[:, :, :, HS:128], 0.0)
            nc.vector.memset(v2[:, :, :, HS:HS + 1], 1.0)
            yTu = const.tile([128, 4, T], BF16)   # unnormalized y^T
            den_dram = dram.tile([8, T], F32)

            with tc.tile_pool(name="mm", bufs=2, space="PSUM") as pmm, \
                 tc.tile_pool(name="pys", bufs=1, space="PSUM") as pys, \
                 tc.tile_pool(name="pss", bufs=2, space="PSUM") as pss, \
                 tc.tile_pool(name="sexp", bufs=6) as sexp, \
                 tc.tile_pool(name="nbc", bufs=2) as nbc, \
                 tc.tile_pool(name="ost", bufs=3) as ostage:

                def qkv_m_group(n, m):
                    ps = pmm.tile([128, QB], F32, tag="mm")
                    for c in range(CK):
                        nc.tensor.matmul(ps,
                                         lhsT=w_qkv[:, c, m * 128:(m + 1) * 128],
                                         rhs=xT[:, c, n * QB:(n + 1) * QB],
                                         start=(c == 0), stop=(c == CK - 1))
                    nc.vector.tensor_scalar_add(out=qkT[:, m, n * QB:(n + 1) * QB],
                                                in0=ps, scalar1=b_qk[:, m:m + 1])

                def qkv_t_group(t):
                    ps = pmm.tile([128, QB], F32, tag="mm")
                    for c in range(CK):
                        nc.tensor.matmul(ps,
                                         lhsT=xT[:, c, t * 128:(t + 1) * 128],
                                         rhs=w_qkv[:, c, 2 * GC:3 * GC],
                                         start=(c == 0), stop=(c == CK - 1))
                    nc.vector.tensor_tensor(out=v2[:, t, :, 0:HS],
                                            in0=ps.rearrange("p (h d) -> p h d", d=HS),
                                            in1=b_v.rearrange("p (h d) -> p h d", d=HS),
                                            op=ALU.add)

                def qkv_groups(n):
                    gs = [(lambda n=n, m=m: qkv_m_group(n, m)) for m in range(8)]
                    gs += [(lambda t=t: qkv_t_group(t)) for t in range(4 * n, 4 * n + 4)]
                    return gs

                def proj_group(t, n2):
                    ps = pmm.tile([128, QB], F32, tag="mm")
                    for c4 in range(4):
                        nc.tensor.matmul(ps,
                                         lhsT=yTu[:, c4, t * 128:(t + 1) * 128],
                                         rhs=w_proj[:, c4, n2 * QB:(n2 + 1) * QB],
                                         start=(c4 == 0), stop=(c4 == 3))
                    ot = ostage.tile([128, QB], F32, tag="ot")
                    nc.vector.tensor_tensor(out=ot, in0=ps,
                                            in1=b_proj[:, n2 * QB:(n2 + 1) * QB],
                                            op=ALU.add)
                    eng = nc.sync if (2 * t + n2) % 2 == 0 else nc.scalar
                    eng.dma_start(out=y_d[t * 128:(t + 1) * 128, n2 * QB:(n2 + 1) * QB],
                                  in_=ot)

                def proj_groups(qj):
                    return [(lambda t=t, n2=n2: proj_group(t, n2))
                            for t in range(4 * qj, 4 * qj + 4) for n2 in range(2)]

                def attn_chunk(qj, hp):
                    nki = 4 * (qj + 1)
                    qcols = slice(qj * QB, (qj + 1) * QB)
                    py0 = pys.tile([128, QB], F32, tag="py0")
                    py1 = pys.tile([128, QB], F32, tag="py1")
                    for ki in range(nki):
                        d = ki - 4 * qj
                        q0 = max(d, 0) * 128
                        cols = slice(qj * QB + q0, (qj + 1) * QB)
                        ps = pss.tile([128, 2, QB], F32, tag="s")
                        # head pair on disjoint PE row-groups -> concurrent
                        nc.tensor.matmul(ps[:, 0, q0:],
                                         lhsT=qkT[0:HS, 4 + hp, ki * 128:(ki + 1) * 128],
                                         rhs=qkT[0:HS, hp, cols],
                                         start=True, stop=True)
                        nc.tensor.matmul(ps[:, 1, q0:],
                                         lhsT=qkT[HS:128, 4 + hp, ki * 128:(ki + 1) * 128],
                                         rhs=qkT[HS:128, hp, cols],
                                         start=True, stop=True)
                        ex = sexp.tile([128, 2, QB], BF16, tag="e")
                        nc.scalar.activation(out=ex[:, :, q0:], in_=ps[:, :, q0:],
                                             func=AF.Exp, scale=SCALE)
                        if d >= 0:
                            nc.vector.tensor_tensor(out=ex[:, :, q0:q0 + 128],
                                                    in0=ex[:, :, q0:q0 + 128],
                                                    in1=mask2, op=ALU.mult)
                        nc.tensor.matmul(py0[:, q0:], lhsT=v2[:, ki, 2 * hp, :],
                                         rhs=ex[:, 0, q0:],
                                         start=(ki == 0), stop=(ki == nki - 1))
                        nc.tensor.matmul(py1[:, q0:], lhsT=v2[:, ki, 2 * hp + 1, :],
                                         rhs=ex[:, 1, q0:],
                                         start=(ki == 0), stop=(ki == nki - 1))
                    # drain: y rows (0..63) to yTu (DVE straight + ACT
                    # shifted), colsum rows via DVE at straight base-64
                    cst = nbc.tile([65, 2, QB], F32, tag="cs")
                    nc.vector.tensor_copy(out=yTu[0:HS, hp, qcols], in_=py0[0:HS, :])
                    nc.scalar.copy(out=yTu[HS:128, hp, qcols], in_=py1[0:HS, :])
                    nc.vector.tensor_copy(out=cst[HS:HS + 1, 0, :], in_=py0[HS:HS + 1, :])
                    nc.vector.tensor_copy(out=cst[HS:HS + 1, 1, :], in_=py1[HS:HS + 1, :])
                    return cst

                def normalize_hp(qj, hp, cst):
                    # colsums bounce through DRAM to broadcast across
                    # partitions (same-queue FIFO orders store->load); one
                    # reciprocal on all 128 lanes after the broadcast
                    qcols = slice(qj * QB, (qj + 1) * QB)
                    h0, h1 = 2 * hp, 2 * hp + 1
                    nc.sync.dma_start(out=den_dram[h0:h0 + 1, qcols], in_=cst[HS:HS + 1, 0, :])
                    nc.sync.dma_start(out=den_dram[h1:h1 + 1, qcols], in_=cst[HS:HS + 1, 1, :])
                    rd = nbc.tile([128, QB], F32, tag="rd")
                    nc.sync.dma_start(out=rd[0:HS, :],
                                      in_=bcast(den_dram[h0:h0 + 1, qcols], HS))
                    nc.sync.dma_start(out=rd[HS:128, :],
                                      in_=bcast(den_dram[h1:h1 + 1, qcols], HS))
                    rb = nbc.tile([128, QB], F32, tag="rb")
                    nc.vector.reciprocal_approx_fast(out=rb, in_=rd)
                    nc.vector.tensor_tensor(out=yTu[:, hp, qcols],
                                            in0=yTu[:, hp, qcols],
                                            in1=rb, op=ALU.mult)

                # ---------------- interleaved emission ----------------
                for g in qkv_groups(0):
                    g()
                fillers = {
                    0: qkv_groups(1),
                    1: qkv_groups(2) + proj_groups(0),
                    2: qkv_groups(3) + proj_groups(1),
                    3: proj_groups(2),
                }
                for qj in range(NQ):
                    fl = fillers[qj]
                    k = 0
                    for hp in range(4):
                        cst = attn_chunk(qj, hp)
                        normalize_hp(qj, hp, cst)
                        take = (len(fl) * (hp + 1)) // 4 - (len(fl) * hp) // 4
                        for _ in range(take):
                            fl[k]()
                            k += 1
                for g in proj_groups(3):
                    g()

    nc.compile()
    return nc


def make_mask2():
    kk = np.arange(128)[:, None]
    qq = np.arange(128)[None, :]
    m = (qq >= kk).astype(ml_dtypes.bfloat16)
    return np.ascontiguousarray(np.broadcast_to(m[:, None, :], (128, 2, 128)))


def make_in_maps(x, w_attn, b_attn, w_proj, b_proj):
    mask2 = make_mask2()
    in_maps = []
    for core in range(N_CORES):
        b, g = core // 2, core % 2
        cq = slice(g * GC, (g + 1) * GC)
        ck = slice(C + g * GC, C + (g + 1) * GC)
        cv = slice(2 * C + g * GC, 2 * C + (g + 1) * GC)
        w_qkv_g = np.concatenate([w_attn[:, cq], w_attn[:, ck], w_attn[:, cv]], axis=1)
        in_maps.append({
            "xT": np.ascontiguousarray(
                np.asarray(x[b], dtype=np.float32).astype(ml_dtypes.bfloat16).T),
            "w_qkv": np.ascontiguousarray(w_qkv_g.astype(ml_dtypes.bfloat16)),
            # pre-tiled [128, 8]: b_qk[p, m] = flat[m*128 + p] (contiguous DMA)
            "b_qk": np.ascontiguousarray(
                np.concatenate([b_attn[cq], b_attn[ck]]).astype(np.float32)
                .reshape(8, 128).T),
            "b_v": np.ascontiguousarray(b_attn[cv]).astype(np.float32),
            "w_proj": np.ascontiguousarray(w_proj[g * GC:(g + 1) * GC, :].astype(ml_dtypes.bfloat16)),
            "b_proj": (b_proj if g == 0 else np.zeros_like(b_proj)).astype(np.float32),
            "mask2": mask2,
        })
    return in_maps


_PROGRAM = None


def kernel(x, w_attn, b_attn, w_proj, b_proj, _trace=False):
    global _PROGRAM
    x = np.asarray(x)
    B = x.shape[0]
    if _PROGRAM is None:
        _PROGRAM = build_program()
    nc = _PROGRAM
    in_maps = make_in_maps(x, np.asarray(w_attn), np.asarray(b_attn),
                           np.asarray(w_proj), np.asarray(b_proj))
    res = bass_utils.run_bass_kernel_spmd(nc, in_maps, core_ids=list(range(N_CORES)),
                                          trace=_trace)
    y = np.zeros((B, T, C), np.float32)
    for b in range(B):
        y[b] = res.results[2 * b]["y"] + res.results[2 * b + 1]["y"]
    if _trace:
        return y, res
    return y
